# revision 20
# baseline (speedup 1.0000x reference)
"""DA3 CrossFrame CF Angle Loss — Trainium2 Bass kernel (8-core SPMD).

Sharding: sim/topk phase sharded over the 8192 extra rows (E); angle
phase sharded over the 512 ref rows (R). One AllToAll per batch entry
exchanges per-ref top-4 candidates. Per-core partial sums [3] are
combined on the host.

v5: all big matmuls (sim, sr, u1) run in fp8(e4m3) with
perf_mode=DoubleRow — 256-deep contraction per pass, half the passes,
half the load bytes. The transposed dma_gather interleaves fp8 pairs at
16-bit granularity, which is exactly the DoubleRow operand layout, so
the gathered h^T feeds the angle matmuls directly. The -0.5*ss row is
still added with a tiny bf16 stop matmul into the same psum group.
Errors stay tiny because each dot product averages 1024 independent
quantization errors (~1e-3 abs on unit-scale dots).

Structure (from v4): sim inputs split across both HWDGE queues and
loaded first; per-b AllToAll fired as soon as its sim finishes; all sr
matmuls emitted right after sim to cover the collective's cross-core
skew; merge with two-step it16 wrap + replications on the by-then-idle
load queues; gathers issued per-b immediately; 6 round-robin psum tags
for the u1 chains.
"""

import numpy as np
import ml_dtypes

import concourse.bass as bass
import concourse.bacc as bacc
import concourse.mybir as mybir
import concourse.bass_isa as bass_isa
from concourse.tile import TileContext
from concourse.bass_utils import run_bass_kernel_spmd
from concourse import library_config

F32 = mybir.dt.float32
BF16 = mybir.dt.bfloat16
FP8 = mybir.dt.float8e4
I16 = mybir.dt.int16
I32 = mybir.dt.int32
U32 = mybir.dt.uint32
AF = mybir.ActivationFunctionType
OP = mybir.AluOpType
AX = mybir.AxisListType
DR = mybir.MatmulPerfMode.DoubleRow
DRS = mybir.MatmulPerfMode.DoubleRowSwInterleave

NC_N = 8
B, P, D = 2, 2048, 1024
R = S = 512
K = 4
E = 4 * P          # 8192
ESH = E // NC_N    # 1024
RSH = R // NC_N    # 64
RK = RSH * K       # 256  (j = k*RSH + r, k outer)
DC2 = D // 256     # 4 double-row contraction blocks
NFP = 3            # frame pairs


def raw_activation(nc, out, in_, func, bias=0.0, scale=1.0, accum_out=None):
    """nc.scalar.activation without the Rsqrt/Reciprocal accuracy guard."""
    se = nc.scalar
    if isinstance(bias, float):
        bias = nc.const_aps.scalar_like(bias, in_)
    inputs = [se.lower_ap(in_)]
    for arg in (bias, scale, 0.0):
        if isinstance(arg, bass.AP):
            inputs.append(se.lower_ap(arg))
        else:
            inputs.append(mybir.ImmediateValue(dtype=mybir.dt.float32, value=arg))
    outputs = [se.lower_ap(out)]
    if accum_out is not None:
        outputs.append(se.lower_ap(accum_out))
    return se.add_instruction(
        mybir.InstActivation(
            name=nc.get_next_instruction_name(),
            func=func,
            ins=inputs,
            outs=outputs,
        )
    )


def build(debug=False):
    nc = bacc.Bacc("TRN2", target_bir_lowering=False, debug=False,
                   num_devices=NC_N)

    T = {}
    T["extTn"] = nc.dram_tensor("extTn", [B, 128, DC2 * 2 * ESH], FP8, kind="ExternalInput")
    T["refTt"] = nc.dram_tensor("refTt", [B, 128, DC2 * 2 * R], FP8, kind="ExternalInput")
    T["refoT"] = nc.dram_tensor("refoT", [2, B, 128, DC2 * 2 * 128], FP8, kind="ExternalInput")
    T["refnat"] = nc.dram_tensor("refnat", [2, B, 128, D], BF16, kind="ExternalInput")
    T["shT"] = nc.dram_tensor("shT", [NFP, 2, B, 128, DC2 * 2 * S], FP8, kind="ExternalInput")
    T["extrows"] = nc.dram_tensor("extrows", [B, E, D], BF16, kind="ExternalInput")
    T["extrows8"] = nc.dram_tensor("extrows8", [B, E, D], FP8, kind="ExternalInput")
    T["ss12"] = nc.dram_tensor("ss12", [2, 12 * 512], BF16, kind="ExternalInput")
    T["mhalf"] = nc.dram_tensor("mhalf", [2, 128], BF16, kind="ExternalInput")
    T["offtab"] = nc.dram_tensor("offtab", [128, 32], I32, kind="ExternalInput")
    T["rrep"] = nc.dram_tensor("rrep", [128, 4], F32, kind="ExternalInput")
    T["partials"] = nc.dram_tensor("partials", [1, 4], F32, kind="ExternalOutput")

    with TileContext(nc) as tc:
        _body(nc, tc, T)
    nc.compile()
    return nc


def _body(nc, tc, T):
    extTn_d, refTt_d, refoT_d = T["extTn"], T["refTt"], T["refoT"]
    refnat_d, shT_d = T["refnat"], T["shT"]
    extrows_d, extrows8_d = T["extrows"], T["extrows8"]
    ss12_d, offtab_d, rrep_d = T["ss12"], T["offtab"], T["rrep"]
    partials = T["partials"]

    with tc.tile_pool(name="con", bufs=1) as con, \
         tc.tile_pool(name="ps", bufs=1, space="PSUM") as psp, \
         tc.tile_pool(name="dram", bufs=1, space="DRAM") as dram:

        nc.gpsimd.load_library(library_config.mlp)

        # ---------- tiny constants (scalar queue, instant) ----------
        oft_sb = con.tile([128, 32], I32, name="oft", tag="oft")
        nc.scalar.dma_start(out=oft_sb[:], in_=offtab_d[:])
        mh_sb = con.tile([2, 128], BF16, name="mh", tag="mh")
        nc.scalar.dma_start(out=mh_sb[:], in_=T["mhalf"][:])
        rrep_sb = con.tile([128, 4], F32, name="rrep", tag="rrep")
        nc.scalar.dma_start(out=rrep_sb[:], in_=rrep_d[:])
        ss12_sb = con.tile([2, 12 * 512], BF16, name="ss12", tag="ss12")
        nc.scalar.dma_start(out=ss12_sb[:], in_=ss12_d[:])

        # ---------- sim inputs first, split across both queues ----------
        p1 = tc.alloc_tile_pool(name="p1", bufs=1)
        refTt_sb = []
        extTn_sb = []   # [b][half], half = q 0-1 / 2-3, tile [128, 2*2*ESH]
        for b in range(B):
            eng = nc.sync if b == 0 else nc.scalar
            rt = p1.tile([128, DC2 * 2 * R], FP8, name=f"refTt{b}", tag=f"refTt{b}")
            eng.dma_start(out=rt[:], in_=refTt_d[b])
            refTt_sb.append(rt)
            halves = []
            for h in range(2):
                et = p1.tile([128, 2 * 2 * ESH], FP8, name=f"extTn{b}{h}",
                             tag=f"extTn{b}{h}")
                eng.dma_start(out=et[:],
                              in_=extTn_d[b, :, h * 2 * 2 * ESH:(h + 1) * 2 * 2 * ESH])
                halves.append(et)
            extTn_sb.append(halves)

        # ---------- shT (xi0 sync / xi1 scalar), refoT ----------
        shT_sb = [dict() for _ in range(NFP)]

        def load_shT(f, pool):
            for xi in range(2):
                eng = nc.sync if xi == 0 else nc.scalar
                for b in range(B):
                    tl = pool.tile([128, DC2 * 2 * S], FP8, name=f"sh{f}{xi}{b}",
                                   tag=f"sh{f}{xi}{b}")
                    eng.dma_start(out=tl[:], in_=shT_d[f, xi, b])
                    shT_sb[f][(xi, b)] = tl

        load_shT(0, con)
        refoT_sb = {}
        for xi in range(2):
            for b in range(B):
                t1_ = con.tile([128, DC2 * 2 * 128], FP8, name=f"refoT{xi}{b}",
                               tag=f"refoT{xi}{b}")
                nc.scalar.dma_start(out=t1_[:], in_=refoT_d[xi, b])
                refoT_sb[(xi, b)] = t1_
        load_shT(1, con)
        load_shT(2, con)

        # ---------- persistent angle-phase tiles ----------
        scal = con.tile([128, 64], F32, name="scal", tag="scal")
        acc = con.tile([128, 20], F32, name="acc", tag="acc")
        it_sb = [con.tile([128, 16], I16, name=f"it{b}", tag=f"it{b}") for b in range(B)]
        it16 = [con.tile([16, 16], I16, name=f"it16{b}", tag=f"it16{b}") for b in range(B)]
        it_sbR = [con.tile([128, 16], I16, name=f"itR{b}", tag=f"itR{b}") for b in range(B)]
        it16R = [con.tile([16, 16], I16, name=f"it16R{b}", tag=f"it16R{b}") for b in range(B)]

        widx = dram.tile([B, RK], I16, name="widx", tag="widx")
        widxR = dram.tile([B, RK], I16, name="widxR", tag="widxR")
        a2a_in = [dram.tile([NC_N, 512], F32, name=f"a2a_in{b}", tag=f"a2a_in{b}")
                  for b in range(B)]
        a2a_out = [dram.tile([NC_N, 512], F32, name=f"a2a_out{b}", tag=f"a2a_out{b}")
                   for b in range(B)]

        def psum_tile(idx):
            return psp.tile([128, 512], F32, name=f"pp{idx}", tag=f"pp{idx}")

        def dr_view(tile):
            # [128, DC2*2*n] fp8 tile -> [128, q, i, n] DoubleRow operand view
            return tile[:].rearrange("p (q i n) -> p q i n", q=DC2, i=2)

        # ================= phase 1: sim + topk + pack + A2A =================
        sim_sb = [p1.tile([128, ESH], F32, name=f"sim{b}", tag="sim")
                  for b in range(B)]
        cand = [p1.tile([128, 32], F32, name=f"cand{b}", tag=f"cand{b}")
                for b in range(B)]
        for b in range(B):
            for rc in range(4):
                for eh in range(2):
                    ps = psum_tile((rc % 2) * 2 + eh)
                    for q in range(DC2):
                        lhs = dr_view(refTt_sb[b])[:, q, :,
                                                   rc * 128:(rc + 1) * 128]
                        rhs = extTn_sb[b][q // 2][:].rearrange(
                            "p (qq i e) -> p qq i e", qq=2, i=2)[
                            :, q % 2, :, eh * 512:(eh + 1) * 512]
                        nc.tensor.matmul(ps[:], lhs, rhs,
                                         start=(q == 0), stop=(q == DC2 - 1),
                                         perf_mode=DR)
                    nc.scalar.activation(sim_sb[b][:, eh * 512:(eh + 1) * 512],
                                         ps[:], AF.Copy)
                mxt = p1.tile([128, 8], F32, name="mx", tag=f"mx{b}{rc}")
                mit = p1.tile([128, 8], U32, name="mi", tag=f"mi{b}{rc}")
                nc.vector.max(out=mxt[:], in_=sim_sb[b][:])
                nc.vector.max_index(out=mit[:], in_max=mxt[:], in_values=sim_sb[b][:])
                nc.vector.tensor_copy(cand[b][:, rc * 8:rc * 8 + 4], mxt[:, 0:4])
                nc.vector.tensor_copy(cand[b][:, rc * 8 + 4:rc * 8 + 8].bitcast(U32),
                                      mit[:, 0:4])
            # pack + exchange this b immediately (single DMA on gpsimd queue)
            nc.gpsimd.dma_start(
                out=a2a_in[b][:].rearrange("(rc ph) (r c) -> (ph r) rc c",
                                           ph=2, c=8),
                in_=cand[b][:].rearrange("p (rc c) -> p rc c", c=8))
            nc.gpsimd.collective_compute(
                "AllToAll", OP.bypass, replica_groups=[list(range(NC_N))],
                ins=[a2a_in[b][:]], outs=[a2a_out[b][:]])

        p1.release()

        # ---------- deferred pools (reuse p1 space) ----------
        with tc.tile_pool(name="p2", bufs=1) as p2, \
             tc.tile_pool(name="drn", bufs=1) as drn, \
             tc.tile_pool(name="p5", bufs=2) as p5, \
             tc.tile_pool(name="p5a", bufs=1) as p5a, \
             tc.tile_pool(name="p5b", bufs=2) as p5b:

            go = [p2.tile([128, 2, D], BF16, name=f"go{b}", tag=f"go{b}")
                  for b in range(B)]
            hT = [p2.tile([128, 2 * D], FP8, name=f"hT{b}", tag=f"hT{b}")
                  for b in range(B)]
            refnat_sb = {}
            for xi in range(2):
                for b in range(B):
                    t2_ = p2.tile([128, D], BF16, name=f"refnat{xi}{b}",
                                  tag=f"refnat{xi}{b}")
                    nc.scalar.dma_start(out=t2_[:], in_=refnat_d[xi, b])
                    refnat_sb[(xi, b)] = t2_

            # ---------- sr matmuls for ALL f (fill the A2A skew window) ------
            sr_drained = {}
            for f in range(NFP):
                for xi in range(2):
                    sr2 = drn.tile([128, 1024], BF16, name="sr2", tag=f"sr2_{f}_{xi}")
                    isr = drn.tile([128, 1024], BF16, name="isr", tag=f"isr_{f}_{xi}")
                    sr_drained[(f, xi)] = (sr2, isr)

            def emit_sr(f):
                for xi in range(2):
                    sr2, isr = sr_drained[(f, xi)]
                    for b in range(B):
                        blk = (f * 2 + xi) * 2 + b
                        ss_row = ss12_sb[0:2, blk * 512:(blk + 1) * 512]
                        ps = psum_tile(6 + b)
                        for q in range(DC2):
                            nc.tensor.matmul(
                                ps[:],
                                dr_view(refoT_sb[(xi, b)])[:, q],
                                dr_view(shT_sb[f][(xi, b)])[:, q],
                                start=(q == 0), stop=False, perf_mode=DR)
                        nc.tensor.matmul(ps[:], mh_sb[:], ss_row, start=False,
                                         stop=True)
                        sl = slice(b * 512, (b + 1) * 512)
                        nc.scalar.activation(sr2[:, sl], ps[:], AF.Copy)
                        raw_activation(nc, isr[:, sl], ps[:], AF.Rsqrt,
                                       bias=rrep_sb[:, xi * 2 + b:xi * 2 + b + 1],
                                       scale=-2.0)

            emit_sr(0)
            emit_sr(1)
            emit_sr(2)

            # ---------- merge per b (vector), gathers immediately ----------
            for b in range(B):
                vi_b = con.tile([64, 64], F32, name=f"vi{b}", tag=f"vi{b}")
                nc.gpsimd.dma_start(
                    out=vi_b[:].rearrange("p (j c) -> p j c", c=8),
                    in_=a2a_out[b][:].rearrange("j (r c) -> r j c", c=8))
                vi3 = vi_b[:].rearrange("p (j c) -> p j c", c=8)
                v32 = con.tile([64, 32], F32, name=f"v32{b}", tag=f"v32{b}")
                nc.vector.tensor_copy(v32[:].rearrange("p (j k) -> p j k", k=4),
                                      vi3[:, :, 0:4])
                gidx = con.tile([64, 32], I32, name=f"gidx{b}", tag=f"gidx{b}")
                nc.vector.tensor_tensor(
                    gidx[:].rearrange("p (j k) -> p j k", k=4),
                    vi3[:, :, 4:8].bitcast(I32),
                    oft_sb[0:64, :].rearrange("p (j k) -> p j k", k=4), OP.add)
                gidxf = con.tile([64, 32], F32, name=f"gidxf{b}", tag=f"gidxf{b}")
                nc.vector.tensor_copy(gidxf[:], gidx[:])
                mv = con.tile([64, 8], F32, name=f"mv{b}", tag=f"mv{b}")
                nc.vector.max(out=mv[:], in_=v32[:])
                winf_b = con.tile([64, 4], F32, name=f"winf{b}", tag=f"winf{b}")
                for k in range(K):
                    msk = p5.tile([64, 32], F32, name="msk", tag="msk")
                    nc.vector.tensor_scalar(msk[:], v32[:], mv[:, k:k + 1], None,
                                            OP.is_equal)
                    junkC = p5.tile([64, 32], F32, name="junkC", tag="junkC")
                    nc.vector.scalar_tensor_tensor(junkC[:], gidxf[:], 0.0, msk[:],
                                                   OP.add, OP.mult,
                                                   accum_out=winf_b[:, k:k + 1])
                win16_b = con.tile([64, 4], I16, name=f"win16{b}", tag=f"win16{b}")
                nc.vector.tensor_copy(win16_b[:], winf_b[:])
                # DRAM bounce: widx[t] holds idx of h-row j=t. Forward wrap
                # feeds go's gather; a DRAM->DRAM block-reversed copy feeds
                # hT's gather (forward-AP SwInterleave then un-reverses).
                nc.gpsimd.dma_start(
                    out=widx[b].rearrange("(k r) -> r k", k=4),
                    in_=win16_b[:])
                nc.gpsimd.dma_start(
                    out=widxR[b].rearrange("(blk t) -> blk t", t=128),
                    in_=widx[b].rearrange("(blk t) -> blk t", t=128)[:, ::-1])
                nc.gpsimd.dma_start(
                    out=it16[b][:],
                    in_=widx[b].rearrange("(c p) -> p c", p=16))
                nc.gpsimd.dma_start(
                    out=it16R[b][:],
                    in_=widxR[b].rearrange("(c p) -> p c", p=16))
                for rep in range(NC_N):
                    eng = nc.sync if rep % 2 == 0 else nc.scalar
                    eng.dma_start(out=it_sb[b][rep * 16:(rep + 1) * 16, :],
                                  in_=it16[b][:])
                    eng.dma_start(out=it_sbR[b][rep * 16:(rep + 1) * 16, :],
                                  in_=it16R[b][:])
                nc.gpsimd.dma_gather(go[b][:], extrows_d[b], it_sb[b][:], RK, RK, D,
                                     single_packet=True)
                nc.gpsimd.dma_gather(
                    hT[b][:].rearrange("p (a j) -> p a j", a=8),
                    extrows8_d[b], it_sbR[b][:], RK, RK, D,
                    transpose=True, single_packet=False)

            # ---------- scal blocks (after both merges) ----------
            # scal columns: 0-3 hh(b,c) | 4-7 hh/2 | 8-15 rh(xi,b,c) | 16-23 ih |
            # 24-31 nih | 32-39 cih2 | 40-47 dih2 | 48-55 d' | 56-63 -rh
            for b in range(B):
                for c in range(2):
                    junkB = p5.tile([128, D], BF16, name="junkB", tag="junkB", bufs=1)
                    nc.scalar.activation(junkB[:], go[b][:, c, :], AF.Square,
                                         accum_out=scal[:, b * 2 + c:b * 2 + c + 1])
                for xi in range(2):
                    for c in range(2):
                        col = 8 + xi * 4 + b * 2 + c
                        junkB = p5.tile([128, D], BF16, name="junkB", tag="junkB", bufs=1)
                        nc.vector.scalar_tensor_tensor(
                            junkB[:], go[b][:, c, :], 1.0, refnat_sb[(xi, b)][:],
                            OP.bypass, OP.mult, accum_out=scal[:, col:col + 1])
                hh2 = scal[:, b * 2:b * 2 + 2]
                hhh = scal[:, 4 + b * 2:6 + b * 2]
                nc.vector.tensor_scalar_mul(hhh, hh2, 0.5)
                for xi in range(2):
                    o = xi * 4 + b * 2
                    rh2 = scal[:, 8 + o:10 + o]
                    ih2 = scal[:, 16 + o:18 + o]
                    nih2 = scal[:, 24 + o:26 + o]
                    cih2 = scal[:, 32 + o:34 + o]
                    dih2 = scal[:, 40 + o:42 + o]
                    dp2 = scal[:, 48 + o:50 + o]
                    rrbc = rrep_sb[:, xi * 2 + b:xi * 2 + b + 1].to_broadcast([128, 2])
                    t1 = p5.tile([128, 2], F32, name="t1", tag="t1")
                    nc.vector.tensor_scalar_mul(t1[:], rh2, -2.0)
                    nc.vector.tensor_add(t1[:], t1[:], hh2)
                    t2 = p5.tile([128, 2], F32, name="t2", tag="t2")
                    nc.vector.tensor_tensor(t2[:], t1[:], rrbc, OP.add)  # nhr^2
                    nhr = p5.tile([128, 2], F32, name="nhr", tag="nhr")
                    nc.scalar.activation(nhr[:], t2[:], AF.Sqrt)
                    nc.vector.reciprocal(ih2, nhr[:])
                    nc.vector.tensor_scalar_mul(nih2, ih2, -1.0)
                    t3 = p5.tile([128, 2], F32, name="t3", tag="t3")
                    nc.vector.tensor_tensor(t3[:], rh2, rrbc, OP.subtract)  # rh-rr
                    nc.vector.tensor_sub(t3[:], hhh, t3[:])  # hh/2-rh+rr
                    nc.vector.tensor_mul(cih2, t3[:], ih2)
                    nc.vector.tensor_sub(dp2, hhh, rh2)      # d' = hh/2-rh
                    nc.vector.tensor_mul(dih2, dp2, ih2)
                    # cih2 += nih*hh/2 ; dih2 += ih*hh/2 ; nrh = -rh
                    t4 = p5.tile([128, 2], F32, name="t4", tag="t4")
                    nc.vector.tensor_mul(t4[:], nih2, hhh)
                    nc.vector.tensor_add(cih2, cih2, t4[:])
                    nc.vector.tensor_mul(t4[:], ih2, hhh)
                    nc.vector.tensor_add(dih2, dih2, t4[:])
                    nc.vector.tensor_scalar_mul(scal[:, 56 + o:58 + o], rh2, -1.0)

            # ---------- angle grids ----------
            a_t = None
            for f in range(NFP):
                for xi in range(2):
                    sr2, isr = sr_drained[(f, xi)]
                    u1 = [p5b.tile([128, 1024], BF16, name="u1", tag=f"u1_{c}") for c in range(2)]
                    ish = [p5b.tile([128, 1024], BF16, name="ish", tag=f"ish_{c}") for c in range(2)]
                    for b in range(B):
                        blk = (f * 2 + xi) * 2 + b
                        ss_row = ss12_sb[0:2, blk * 512:(blk + 1) * 512]
                        sl = slice(b * 512, (b + 1) * 512)
                        hv = hT[b][:].rearrange("p (q j i) -> p q j i",
                                                q=DC2, j=256, i=2)
                        pss = []
                        for c in range(2):
                            g = (f * 2 + xi) * 4 + b * 2 + c
                            ps = psum_tile(g % 6)
                            for q in range(DC2):
                                nc.tensor.matmul(
                                    ps[:], hv[:, q, c * 128:(c + 1) * 128, :],
                                    dr_view(shT_sb[f][(xi, b)])[:, q],
                                    start=(q == 0), stop=False, perf_mode=DRS)
                            nc.tensor.matmul(ps[:], mh_sb[:], ss_row, start=False,
                                             stop=True)
                            pss.append(ps)
                        for c in range(2):
                            hhc = scal[:, b * 2 + c:b * 2 + c + 1]
                            nc.scalar.activation(u1[c][:, sl], pss[c][:], AF.Copy)
                            raw_activation(nc, ish[c][:, sl], pss[c][:], AF.Rsqrt,
                                           scale=-2.0, bias=hhc)

                    aj = []
                    for c in range(2):
                        t0 = p5a.tile([128, 1024], BF16, name="t0", tag="t0")
                        nc.vector.tensor_sub(t0[:], sr2[:], u1[c][:])
                        t1 = p5a.tile([128, 1024], BF16, name="t1g", tag="t1g")
                        nc.vector.tensor_add(t1[:], sr2[:], u1[c][:])
                        pp = p5a.tile([128, 1024], BF16, name="pp", tag="ppg")
                        nc.vector.tensor_mul(pp[:], isr[:], ish[c][:])
                        q = p5a.tile([128, 1024], BF16, name="q", tag="q")
                        pt = p5a.tile([128, 1024], BF16, name="pt", tag="pt")
                        w1 = p5a.tile([128, 1024], BF16, name="w1", tag="w1")
                        for b in range(B):
                            sl = slice(b * 512, (b + 1) * 512)
                            col = b * 2 + c
                            nc.vector.tensor_scalar(
                                q[:, sl], t0[:, sl],
                                scal[:, 24 + xi * 4 + col:25 + xi * 4 + col],
                                scal[:, 32 + xi * 4 + col:33 + xi * 4 + col],
                                OP.mult, OP.add)
                            nc.vector.tensor_scalar(
                                pt[:, sl], t0[:, sl],
                                scal[:, 16 + xi * 4 + col:17 + xi * 4 + col],
                                scal[:, 40 + xi * 4 + col:41 + xi * 4 + col],
                                OP.mult, OP.add)
                            nc.vector.tensor_scalar(
                                w1[:, sl], t1[:, sl],
                                scal[:, 56 + xi * 4 + col:57 + xi * 4 + col],
                                None, OP.add)
                        if xi == 0:
                            a1 = p5a.tile([128, 1024], BF16, name="a1", tag=f"a1_{c}")
                            nc.vector.tensor_mul(a1[:], q[:], isr[:])
                            a2 = p5a.tile([128, 1024], BF16, name="a2", tag=f"a2_{c}")
                            nc.vector.tensor_mul(a2[:], pt[:], ish[c][:])
                            a3 = p5a.tile([128, 1024], BF16, name="a3", tag=f"a3_{c}")
                            nc.vector.tensor_mul(a3[:], w1[:], pp[:])
                            aj.append((a1, a2, a3))
                        else:
                            # student side: form a_s then diff against teacher
                            srcs = [(q, isr[:]), (pt, ish[c][:]), (w1, pp[:])]
                            for jj in range(3):
                                asrc, mulv = srcs[jj]
                                asb = p5a.tile([128, 1024], BF16, name="as", tag="as")
                                nc.vector.tensor_mul(asb[:], asrc[:], mulv)
                                dj = p5a.tile([128, 1024], BF16, name="dj", tag="dj")
                                nc.vector.tensor_sub(dj[:], asb[:], a_t[c][jj][:])
                                slot = jj * 6 + f * 2 + c
                                if jj == 0:
                                    nc.vector.tensor_reduce(
                                        acc[:, slot:slot + 1], dj[:], AX.X, OP.add,
                                        apply_absolute_value=True)
                                else:
                                    junkB = p5.tile([128, D], BF16, name="junkB",
                                                    tag="junkB", bufs=1)
                                    nc.scalar.activation(junkB[:], dj[:], AF.Abs,
                                                         accum_out=acc[:, slot:slot + 1])
                    if xi == 0:
                        a_t = aj

            # ---------- final ----------
            accr = con.tile([128, 4], F32, name="accr", tag="accr")
            nc.vector.tensor_reduce(accr[:, 0:3],
                                    acc[:, 0:18].rearrange("p (j s) -> p j s", j=3),
                                    AX.X, OP.add)
            nc.vector.memset(accr[:, 3:4], 0.0)
            par = con.tile([128, 4], F32, name="par", tag="par")
            nc.gpsimd.partition_all_reduce(par[:], accr[:], 128,
                                           bass_isa.ReduceOp.add)
            nc.sync.dma_start(out=partials[:], in_=par[0:1, :])


# ---------------- host side ----------------

def bf16(x):
    return np.asarray(x, dtype=ml_dtypes.bfloat16)


def fp8(x):
    return np.asarray(x, dtype=ml_dtypes.float8_e4m3)


def prep_inputs(teacher_feats, student_feats, ref_perm, shared_perm):
    EXTRA_FRAMES = [1, 3, 5, 7]
    tf, sf = np.asarray(teacher_feats), np.asarray(student_feats)
    rp, sp = np.asarray(ref_perm), np.asarray(shared_perm)

    ref = np.stack([tf[:, 0, rp, :], sf[:, 0, rp, :]])          # [2,B,R,D] f32
    ext = np.concatenate([tf[:, f] for f in EXTRA_FRAMES], 1)   # [B,E,D] f32
    sh = np.stack([np.stack([tf[:, t, sp, :], sf[:, s, sp, :]])
                   for s, t in [(1, 2), (2, 4), (3, 6)]])       # [3,2,B,S,D] f32

    extn = ext / np.maximum(np.linalg.norm(ext, axis=-1, keepdims=True), 1e-12)

    def dpack8(x):  # x [..., N, D] f32 -> fp8 [..., 128, DC2*2*N]
        xt = np.swapaxes(x, -1, -2)                             # [..., D, N]
        shp = xt.shape[:-2]
        n = xt.shape[-1]
        xt = xt.reshape(*shp, DC2, 128, 2, n)                   # d = q*256+k*2+i
        xt = np.swapaxes(xt, -4, -3)                            # [..., 128, q, 2, n]
        return np.ascontiguousarray(fp8(xt.reshape(*shp, 128, DC2 * 2 * n)))

    extn_p = dpack8(extn)                                       # [B,128,DC2*2*E]
    refTt_p = dpack8(ref[0])                                    # [B,128,DC2*2*R]
    shT_p = dpack8(sh)                                          # [3,2,B,128,DC2*2*S]

    ss = np.sum(sh.astype(np.float64) * sh, axis=-1)            # [3,2,B,S]
    ss12 = np.zeros((2, 12 * 512), dtype=ml_dtypes.bfloat16)
    ss12[0] = bf16(ss.reshape(-1))
    rrf = np.sum(ref.astype(np.float64) * ref, axis=-1)         # [2,B,R]

    mhalf = np.zeros((2, 128), dtype=ml_dtypes.bfloat16)
    mhalf[0] = -0.5
    offtab = np.broadcast_to((np.arange(32) // 4 * ESH).astype(np.int32),
                             (128, 32)).copy()

    extb = bf16(ext)
    ext8 = fp8(ext)
    in_maps = []
    for c in range(NC_N):
        rs = slice(c * RSH, (c + 1) * RSH)
        esl = slice(c * ESH, (c + 1) * ESH)
        # extTn shard: e-slice within each (q,i) block
        extn_sh = (extn_p.reshape(B, 128, DC2 * 2, E)[:, :, :, esl]
                   .reshape(B, 128, DC2 * 2 * ESH))
        refo = ref[:, :, rs, :]                                  # [2,B,64,D]
        reps = np.concatenate([refo, refo], axis=2)              # [2,B,128,D]
        refoT = dpack8(reps)                                     # [2,B,128,DC2*2*128]
        rrep = np.ascontiguousarray(
            np.concatenate([rrf[:, :, rs], rrf[:, :, rs]], axis=2)  # [2,B,128]
            .reshape(4, 128).T.astype(np.float32))               # [128,4] col=xi*2+b
        m = {
            "extTn": np.ascontiguousarray(extn_sh),
            "refTt": refTt_p,
            "refoT": refoT,
            "refnat": bf16(reps),
            "shT": shT_p,
            "extrows": extb,
            "extrows8": ext8,
            "ss12": ss12, "mhalf": mhalf, "offtab": offtab, "rrep": rrep,
        }
        in_maps.append(m)
    return in_maps


_NC_CACHE = {}


def kernel(teacher_feats, student_feats, ref_perm, shared_perm,
           debug=False, trace=False, use_sim=False):
    key = ("nc", debug)
    if key not in _NC_CACHE:
        _NC_CACHE[key] = build(debug=debug)
    nc = _NC_CACHE[key]
    in_maps = prep_inputs(teacher_feats, student_feats, ref_perm, shared_perm)
    if use_sim:
        from concourse.bass_interp import MultiCoreSim
        nc.insert_bir_kernel_barrier_sem_inc()
        sim = MultiCoreSim(nc, NC_N)
        for t in range(NC_N):
            for name, arr in in_maps[t].items():
                sim.cores[t].tensor(name)[:] = arr
        sim.simulate()
        results = [{"partials": np.array(sim.cores[t].tensor("partials"))}
                   for t in range(NC_N)]

        class _R:
            pass
        res = _R()
        res.results = results
        res.exec_time_ns = None
    else:
        res = run_bass_kernel_spmd(nc, in_maps, list(range(NC_N)), trace=trace)
    parts = np.stack([res.results[c]["partials"][0, :3] for c in range(NC_N)])
    total = B * R * S * K * 3
    loss = np.float32(parts.sum() / total)
    if debug or trace:
        return loss, res
    return loss


# revision 22
# speedup vs baseline: 1.6270x; 1.6270x over previous
"""DA3 CrossFrame CF Angle Loss — Trainium2 Bass kernel (8-core SPMD).

Sharding: sim/topk phase sharded over the 8192 extra rows (E); angle
phase sharded over the 512 ref rows (R). One AllToAll per batch entry
exchanges per-ref top-4 candidates. Per-core partial sums [3] are
combined on the host.

v5: all big matmuls (sim, sr, u1) run in fp8(e4m3) with
perf_mode=DoubleRow — 256-deep contraction per pass, half the passes,
half the load bytes. The transposed dma_gather interleaves fp8 pairs at
16-bit granularity, which is exactly the DoubleRow operand layout, so
the gathered h^T feeds the angle matmuls directly. The -0.5*ss row is
still added with a tiny bf16 stop matmul into the same psum group.
Errors stay tiny because each dot product averages 1024 independent
quantization errors (~1e-3 abs on unit-scale dots).

Structure (from v4): sim inputs split across both HWDGE queues and
loaded first; per-b AllToAll fired as soon as its sim finishes; all sr
matmuls emitted right after sim to cover the collective's cross-core
skew; merge with two-step it16 wrap + replications on the by-then-idle
load queues; gathers issued per-b immediately; 6 round-robin psum tags
for the u1 chains.
"""

import numpy as np
import ml_dtypes

import concourse.bass as bass
import concourse.bacc as bacc
import concourse.mybir as mybir
import concourse.bass_isa as bass_isa
from concourse.tile import TileContext
from concourse.bass_utils import run_bass_kernel_spmd
from concourse import library_config

F32 = mybir.dt.float32
BF16 = mybir.dt.bfloat16
FP8 = mybir.dt.float8e4
I16 = mybir.dt.int16
I32 = mybir.dt.int32
U32 = mybir.dt.uint32
AF = mybir.ActivationFunctionType
OP = mybir.AluOpType
AX = mybir.AxisListType
DR = mybir.MatmulPerfMode.DoubleRow
DRS = mybir.MatmulPerfMode.DoubleRowSwInterleave

NC_N = 8
B, P, D = 2, 2048, 1024
R = S = 512
K = 4
E = 4 * P          # 8192
ESH = E // NC_N    # 1024
RSH = R // NC_N    # 64
RK = RSH * K       # 256  (j = k*RSH + r, k outer)
DC2 = D // 256     # 4 double-row contraction blocks
NFP = 3            # frame pairs


def raw_activation(nc, out, in_, func, bias=0.0, scale=1.0, accum_out=None):
    """nc.scalar.activation without the Rsqrt/Reciprocal accuracy guard."""
    se = nc.scalar
    if isinstance(bias, float):
        bias = nc.const_aps.scalar_like(bias, in_)
    inputs = [se.lower_ap(in_)]
    for arg in (bias, scale, 0.0):
        if isinstance(arg, bass.AP):
            inputs.append(se.lower_ap(arg))
        else:
            inputs.append(mybir.ImmediateValue(dtype=mybir.dt.float32, value=arg))
    outputs = [se.lower_ap(out)]
    if accum_out is not None:
        outputs.append(se.lower_ap(accum_out))
    return se.add_instruction(
        mybir.InstActivation(
            name=nc.get_next_instruction_name(),
            func=func,
            ins=inputs,
            outs=outputs,
        )
    )


def build(debug=False):
    nc = bacc.Bacc("TRN2", target_bir_lowering=False, debug=False,
                   num_devices=NC_N)

    T = {}
    T["extTn"] = nc.dram_tensor("extTn", [B, 128, DC2 * 2 * ESH], FP8, kind="ExternalInput")
    T["refTt"] = nc.dram_tensor("refTt", [B, 128, DC2 * 2 * R], FP8, kind="ExternalInput")
    T["refoT"] = nc.dram_tensor("refoT", [2, B, 128, DC2 * 2 * 128], FP8, kind="ExternalInput")
    T["refnat"] = nc.dram_tensor("refnat", [2, B, 128, D], BF16, kind="ExternalInput")
    T["shT"] = nc.dram_tensor("shT", [NFP, 2, B, 128, DC2 * 2 * S], FP8, kind="ExternalInput")
    T["extrows"] = nc.dram_tensor("extrows", [B, E, D], BF16, kind="ExternalInput")
    T["extrows8"] = nc.dram_tensor("extrows8", [B, E, D], FP8, kind="ExternalInput")
    T["ss12"] = nc.dram_tensor("ss12", [2, 12 * 512], BF16, kind="ExternalInput")
    T["mhalf"] = nc.dram_tensor("mhalf", [2, 128], BF16, kind="ExternalInput")
    T["offtab"] = nc.dram_tensor("offtab", [128, 32], I32, kind="ExternalInput")
    T["rrep"] = nc.dram_tensor("rrep", [128, 4], F32, kind="ExternalInput")
    T["rrepR"] = nc.dram_tensor("rrepR", [128, 4], F32, kind="ExternalInput")
    T["jrev"] = nc.dram_tensor("jrev", [128, 128], F32, kind="ExternalInput")
    T["partials"] = nc.dram_tensor("partials", [1, 4], F32, kind="ExternalOutput")

    with TileContext(nc) as tc:
        _body(nc, tc, T)
    nc.compile()
    return nc


def _body(nc, tc, T):
    extTn_d, refTt_d, refoT_d = T["extTn"], T["refTt"], T["refoT"]
    refnat_d, shT_d = T["refnat"], T["shT"]
    extrows_d, extrows8_d = T["extrows"], T["extrows8"]
    ss12_d, offtab_d, rrep_d = T["ss12"], T["offtab"], T["rrep"]
    partials = T["partials"]

    with tc.tile_pool(name="con", bufs=1) as con, \
         tc.tile_pool(name="ps", bufs=1, space="PSUM") as psp, \
         tc.tile_pool(name="dram", bufs=1, space="DRAM") as dram:

        nc.gpsimd.load_library(library_config.mlp)

        # ---------- tiny constants (scalar queue, instant) ----------
        oft_sb = con.tile([128, 32], I32, name="oft", tag="oft")
        nc.scalar.dma_start(out=oft_sb[:], in_=offtab_d[:])
        mh_sb = con.tile([2, 128], BF16, name="mh", tag="mh")
        nc.scalar.dma_start(out=mh_sb[:], in_=T["mhalf"][:])
        rrep_sb = con.tile([128, 4], F32, name="rrep", tag="rrep")
        nc.scalar.dma_start(out=rrep_sb[:], in_=rrep_d[:])
        rrepR_sb = con.tile([128, 4], F32, name="rrepR", tag="rrepR")
        nc.scalar.dma_start(out=rrepR_sb[:], in_=T["rrepR"][:])
        jrev_sb = con.tile([128, 128], F32, name="jrev", tag="jrev")
        nc.scalar.dma_start(out=jrev_sb[:], in_=T["jrev"][:])
        ss12_sb = con.tile([2, 12 * 512], BF16, name="ss12", tag="ss12")
        nc.scalar.dma_start(out=ss12_sb[:], in_=ss12_d[:])

        # ---------- sim inputs first, split across both queues ----------
        p1 = tc.alloc_tile_pool(name="p1", bufs=1)
        refTt_sb = []
        extTn_sb = []   # [b][half], half = q 0-1 / 2-3, tile [128, 2*2*ESH]
        for b in range(B):
            eng = nc.sync if b == 0 else nc.scalar
            rt = p1.tile([128, DC2 * 2 * R], FP8, name=f"refTt{b}", tag=f"refTt{b}")
            eng.dma_start(out=rt[:], in_=refTt_d[b])
            refTt_sb.append(rt)
            halves = []
            for h in range(2):
                et = p1.tile([128, 2 * 2 * ESH], FP8, name=f"extTn{b}{h}",
                             tag=f"extTn{b}{h}")
                eng.dma_start(out=et[:],
                              in_=extTn_d[b, :, h * 2 * 2 * ESH:(h + 1) * 2 * 2 * ESH])
                halves.append(et)
            extTn_sb.append(halves)

        # ---------- shT (xi0 sync / xi1 scalar), refoT ----------
        shT_sb = [dict() for _ in range(NFP)]

        def load_shT(f, pool):
            for xi in range(2):
                eng = nc.sync if xi == 0 else nc.scalar
                for b in range(B):
                    tl = pool.tile([128, DC2 * 2 * S], FP8, name=f"sh{f}{xi}{b}",
                                   tag=f"sh{f}{xi}{b}")
                    eng.dma_start(out=tl[:], in_=shT_d[f, xi, b])
                    shT_sb[f][(xi, b)] = tl

        load_shT(0, con)
        refoT_sb = {}
        for xi in range(2):
            for b in range(B):
                t1_ = con.tile([128, DC2 * 2 * 128], FP8, name=f"refoT{xi}{b}",
                               tag=f"refoT{xi}{b}")
                nc.scalar.dma_start(out=t1_[:], in_=refoT_d[xi, b])
                refoT_sb[(xi, b)] = t1_
        load_shT(1, con)
        load_shT(2, con)

        # ---------- persistent angle-phase tiles ----------
        scal = con.tile([128, 64], F32, name="scal", tag="scal")
        acc = con.tile([128, 20], F32, name="acc", tag="acc")
        it_sb = [con.tile([128, 16], I16, name=f"it{b}", tag=f"it{b}") for b in range(B)]
        it16 = [con.tile([16, 16], I16, name=f"it16{b}", tag=f"it16{b}") for b in range(B)]

        a2a_in = [dram.tile([NC_N, 512], F32, name=f"a2a_in{b}", tag=f"a2a_in{b}")
                  for b in range(B)]
        a2a_out = [dram.tile([NC_N, 512], F32, name=f"a2a_out{b}", tag=f"a2a_out{b}")
                   for b in range(B)]

        def psum_tile(idx):
            return psp.tile([128, 512], F32, name=f"pp{idx}", tag=f"pp{idx}")

        def dr_view(tile):
            # [128, DC2*2*n] fp8 tile -> [128, q, i, n] DoubleRow operand view
            return tile[:].rearrange("p (q i n) -> p q i n", q=DC2, i=2)

        # ================= phase 1: sim + topk + pack + A2A =================
        sim_sb = [p1.tile([128, ESH], F32, name=f"sim{b}", tag="sim")
                  for b in range(B)]
        cand = [p1.tile([128, 32], F32, name=f"cand{b}", tag=f"cand{b}")
                for b in range(B)]
        for b in range(B):
            for rc in range(4):
                for eh in range(2):
                    ps = psum_tile((rc % 2) * 2 + eh)
                    for q in range(DC2):
                        lhs = dr_view(refTt_sb[b])[:, q, :,
                                                   rc * 128:(rc + 1) * 128]
                        rhs = extTn_sb[b][q // 2][:].rearrange(
                            "p (qq i e) -> p qq i e", qq=2, i=2)[
                            :, q % 2, :, eh * 512:(eh + 1) * 512]
                        nc.tensor.matmul(ps[:], lhs, rhs,
                                         start=(q == 0), stop=(q == DC2 - 1),
                                         perf_mode=DR)
                    nc.scalar.activation(sim_sb[b][:, eh * 512:(eh + 1) * 512],
                                         ps[:], AF.Copy)
                mxt = p1.tile([128, 8], F32, name="mx", tag=f"mx{b}{rc}")
                mit = p1.tile([128, 8], U32, name="mi", tag=f"mi{b}{rc}")
                nc.vector.max(out=mxt[:], in_=sim_sb[b][:])
                nc.vector.max_index(out=mit[:], in_max=mxt[:], in_values=sim_sb[b][:])
                nc.vector.tensor_copy(cand[b][:, rc * 8:rc * 8 + 4], mxt[:, 0:4])
                nc.vector.tensor_copy(cand[b][:, rc * 8 + 4:rc * 8 + 8].bitcast(U32),
                                      mit[:, 0:4])
            # pack + exchange this b immediately (single DMA on gpsimd queue)
            nc.gpsimd.dma_start(
                out=a2a_in[b][:].rearrange("(rc ph) (r c) -> (ph r) rc c",
                                           ph=2, c=8),
                in_=cand[b][:].rearrange("p (rc c) -> p rc c", c=8))
            nc.gpsimd.collective_compute(
                "AllToAll", OP.bypass, replica_groups=[list(range(NC_N))],
                ins=[a2a_in[b][:]], outs=[a2a_out[b][:]])

        p1.release()

        # ---------- deferred pools (reuse p1 space) ----------
        with tc.tile_pool(name="p2", bufs=1) as p2, \
             tc.tile_pool(name="drn", bufs=1) as drn, \
             tc.tile_pool(name="p5", bufs=2) as p5, \
             tc.tile_pool(name="p5a", bufs=1) as p5a, \
             tc.tile_pool(name="p5b", bufs=2) as p5b:

            go = [p2.tile([128, 2, D], BF16, name=f"go{b}", tag=f"go{b}")
                  for b in range(B)]
            hT = [p2.tile([128, 2 * D], FP8, name=f"hT{b}", tag=f"hT{b}")
                  for b in range(B)]
            refnat_sb = {}
            for xi in range(2):
                for b in range(B):
                    t2_ = p2.tile([128, D], BF16, name=f"refnat{xi}{b}",
                                  tag=f"refnat{xi}{b}")
                    nc.scalar.dma_start(out=t2_[:], in_=refnat_d[xi, b])
                    refnat_sb[(xi, b)] = t2_

            # ---------- sr matmuls for ALL f (fill the A2A skew window) ------
            sr_drained = {}
            for f in range(NFP):
                for xi in range(2):
                    sr2 = drn.tile([128, 1024], BF16, name="sr2", tag=f"sr2_{f}_{xi}")
                    isr = drn.tile([128, 1024], BF16, name="isr", tag=f"isr_{f}_{xi}")
                    sr_drained[(f, xi)] = (sr2, isr)

            def emit_sr(f):
                for xi in range(2):
                    sr2, isr = sr_drained[(f, xi)]
                    for b in range(B):
                        blk = (f * 2 + xi) * 2 + b
                        ss_row = ss12_sb[0:2, blk * 512:(blk + 1) * 512]
                        ps = psum_tile(6 + b)
                        for q in range(DC2):
                            nc.tensor.matmul(
                                ps[:],
                                dr_view(refoT_sb[(xi, b)])[:, q],
                                dr_view(shT_sb[f][(xi, b)])[:, q],
                                start=(q == 0), stop=False, perf_mode=DR)
                        nc.tensor.matmul(ps[:], mh_sb[:], ss_row, start=False,
                                         stop=True)
                        sl = slice(b * 512, (b + 1) * 512)
                        nc.scalar.activation(sr2[:, sl], ps[:], AF.Copy)
                        raw_activation(nc, isr[:, sl], ps[:], AF.Rsqrt,
                                       bias=rrepR_sb[:, xi * 2 + b:xi * 2 + b + 1],
                                       scale=-2.0)

            emit_sr(0)
            emit_sr(1)
            emit_sr(2)

            # ---------- merge per b (vector), gathers immediately ----------
            for b in range(B):
                vi_b = con.tile([64, 64], F32, name=f"vi{b}", tag=f"vi{b}")
                nc.gpsimd.dma_start(
                    out=vi_b[:].rearrange("p (j c) -> p j c", c=8),
                    in_=a2a_out[b][:].rearrange("j (r c) -> r j c", c=8))
                vi3 = vi_b[:].rearrange("p (j c) -> p j c", c=8)
                v32 = con.tile([64, 32], F32, name=f"v32{b}", tag=f"v32{b}")
                nc.vector.tensor_copy(v32[:].rearrange("p (j k) -> p j k", k=4),
                                      vi3[:, :, 0:4])
                gidx = con.tile([64, 32], I32, name=f"gidx{b}", tag=f"gidx{b}")
                nc.vector.tensor_tensor(
                    gidx[:].rearrange("p (j k) -> p j k", k=4),
                    vi3[:, :, 4:8].bitcast(I32),
                    oft_sb[0:64, :].rearrange("p (j k) -> p j k", k=4), OP.add)
                gidxf = con.tile([64, 32], F32, name=f"gidxf{b}", tag=f"gidxf{b}")
                nc.vector.tensor_copy(gidxf[:], gidx[:])
                mv = con.tile([64, 8], F32, name=f"mv{b}", tag=f"mv{b}")
                nc.vector.max(out=mv[:], in_=v32[:])
                winf_b = con.tile([64, 4], F32, name=f"winf{b}", tag=f"winf{b}")
                for k in range(K):
                    msk = p5.tile([64, 32], F32, name="msk", tag="msk")
                    nc.vector.tensor_scalar(msk[:], v32[:], mv[:, k:k + 1], None,
                                            OP.is_equal)
                    junkC = p5.tile([64, 32], F32, name="junkC", tag="junkC")
                    nc.vector.scalar_tensor_tensor(junkC[:], gidxf[:], 0.0, msk[:],
                                                   OP.add, OP.mult,
                                                   accum_out=winf_b[:, k:k + 1])
                win16_b = con.tile([64, 4], I16, name=f"win16{b}", tag=f"win16{b}")
                nc.vector.tensor_copy(win16_b[:], winf_b[:])
                # wrap into it16 [16,16] (SBUF->SBUF), then replicate 8x on
                # the now-idle sync/scalar queues
                for rh in range(4):
                    nc.gpsimd.dma_start(
                        out=it16[b][:].rearrange("p (k rh) -> p k rh", rh=4)[:, :, rh],
                        in_=win16_b[rh * 16:(rh + 1) * 16, :])
                for rep in range(NC_N):
                    eng = nc.sync if rep % 2 == 0 else nc.scalar
                    eng.dma_start(out=it_sb[b][rep * 16:(rep + 1) * 16, :],
                                  in_=it16[b][:])
                nc.gpsimd.dma_gather(go[b][:], extrows_d[b], it_sb[b][:], RK, RK, D,
                                     single_packet=True)
                nc.gpsimd.dma_gather(
                    hT[b][:].rearrange("p (a j) -> p a j", a=8),
                    extrows8_d[b], it_sb[b][:], RK, RK, D,
                    transpose=True, single_packet=False)

            # ---------- scal blocks (after both merges) ----------
            # scal columns: 0-3 hh(b,c) | 4-7 hh/2 | 8-15 rh(xi,b,c) | 16-23 ih |
            # 24-31 nih | 32-39 cih2 | 40-47 dih2 | 48-55 d' | 56-63 -rh
            for b in range(B):
                for c in range(2):
                    junkB = p5.tile([128, D], BF16, name="junkB", tag="junkB", bufs=1)
                    nc.scalar.activation(junkB[:], go[b][:, c, :], AF.Square,
                                         accum_out=scal[:, b * 2 + c:b * 2 + c + 1])
                for xi in range(2):
                    for c in range(2):
                        col = 8 + xi * 4 + b * 2 + c
                        junkB = p5.tile([128, D], BF16, name="junkB", tag="junkB", bufs=1)
                        nc.vector.scalar_tensor_tensor(
                            junkB[:], go[b][:, c, :], 1.0, refnat_sb[(xi, b)][:],
                            OP.bypass, OP.mult, accum_out=scal[:, col:col + 1])
                hh2 = scal[:, b * 2:b * 2 + 2]
                hhh = scal[:, 4 + b * 2:6 + b * 2]
                nc.vector.tensor_scalar_mul(hhh, hh2, 0.5)
                for xi in range(2):
                    o = xi * 4 + b * 2
                    rh2 = scal[:, 8 + o:10 + o]
                    ih2 = scal[:, 16 + o:18 + o]
                    nih2 = scal[:, 24 + o:26 + o]
                    cih2 = scal[:, 32 + o:34 + o]
                    dih2 = scal[:, 40 + o:42 + o]
                    dp2 = scal[:, 48 + o:50 + o]
                    rrbc = rrep_sb[:, xi * 2 + b:xi * 2 + b + 1].to_broadcast([128, 2])
                    t1 = p5.tile([128, 2], F32, name="t1", tag="t1")
                    nc.vector.tensor_scalar_mul(t1[:], rh2, -2.0)
                    nc.vector.tensor_add(t1[:], t1[:], hh2)
                    t2 = p5.tile([128, 2], F32, name="t2", tag="t2")
                    nc.vector.tensor_tensor(t2[:], t1[:], rrbc, OP.add)  # nhr^2
                    nhr = p5.tile([128, 2], F32, name="nhr", tag="nhr")
                    nc.scalar.activation(nhr[:], t2[:], AF.Sqrt)
                    nc.vector.reciprocal(ih2, nhr[:])
                    nc.vector.tensor_scalar_mul(nih2, ih2, -1.0)
                    t3 = p5.tile([128, 2], F32, name="t3", tag="t3")
                    nc.vector.tensor_tensor(t3[:], rh2, rrbc, OP.subtract)  # rh-rr
                    nc.vector.tensor_sub(t3[:], hhh, t3[:])  # hh/2-rh+rr
                    nc.vector.tensor_mul(cih2, t3[:], ih2)
                    nc.vector.tensor_sub(dp2, hhh, rh2)      # d' = hh/2-rh
                    nc.vector.tensor_mul(dih2, dp2, ih2)
                    # cih2 += nih*hh/2 ; dih2 += ih*hh/2 ; nrh = -rh
                    t4 = p5.tile([128, 2], F32, name="t4", tag="t4")
                    nc.vector.tensor_mul(t4[:], nih2, hhh)
                    nc.vector.tensor_add(cih2, cih2, t4[:])
                    nc.vector.tensor_mul(t4[:], ih2, hhh)
                    nc.vector.tensor_add(dih2, dih2, t4[:])
                    nc.vector.tensor_scalar_mul(scal[:, 56 + o:58 + o], rh2, -1.0)

            # ---------- partition-reversed scal (aligns with SwInterleave
            # row order of the u1 psums) ----------
            scalR = con.tile([128, 64], F32, name="scalR", tag="scalR")
            psr = psum_tile(6)
            nc.tensor.matmul(psr[:, 0:64], jrev_sb[:], scal[:], start=True,
                             stop=True)
            nc.scalar.activation(scalR[:], psr[:, 0:64], AF.Copy)

            # ---------- angle grids ----------
            a_t = None
            for f in range(NFP):
                for xi in range(2):
                    sr2, isr = sr_drained[(f, xi)]
                    u1 = [p5b.tile([128, 1024], BF16, name="u1", tag=f"u1_{c}") for c in range(2)]
                    ish = [p5b.tile([128, 1024], BF16, name="ish", tag=f"ish_{c}") for c in range(2)]
                    for b in range(B):
                        blk = (f * 2 + xi) * 2 + b
                        ss_row = ss12_sb[0:2, blk * 512:(blk + 1) * 512]
                        sl = slice(b * 512, (b + 1) * 512)
                        hv = hT[b][:].rearrange("p (q j i) -> p q j i",
                                                q=DC2, j=256, i=2)
                        pss = []
                        for c in range(2):
                            g = (f * 2 + xi) * 4 + b * 2 + c
                            ps = psum_tile(g % 6)
                            for q in range(DC2):
                                nc.tensor.matmul(
                                    ps[:], hv[:, q, c * 128:(c + 1) * 128, :],
                                    dr_view(shT_sb[f][(xi, b)])[:, q],
                                    start=(q == 0), stop=False, perf_mode=DRS)
                            nc.tensor.matmul(ps[:], mh_sb[:], ss_row, start=False,
                                             stop=True)
                            pss.append(ps)
                        for c in range(2):
                            hhc = scalR[:, b * 2 + c:b * 2 + c + 1]
                            nc.scalar.activation(u1[c][:, sl], pss[c][:], AF.Copy)
                            raw_activation(nc, ish[c][:, sl], pss[c][:], AF.Rsqrt,
                                           scale=-2.0, bias=hhc)

                    aj = []
                    for c in range(2):
                        t0 = p5a.tile([128, 1024], BF16, name="t0", tag="t0")
                        nc.vector.tensor_sub(t0[:], sr2[:], u1[c][:])
                        t1 = p5a.tile([128, 1024], BF16, name="t1g", tag="t1g")
                        nc.vector.tensor_add(t1[:], sr2[:], u1[c][:])
                        pp = p5a.tile([128, 1024], BF16, name="pp", tag="ppg")
                        nc.vector.tensor_mul(pp[:], isr[:], ish[c][:])
                        q = p5a.tile([128, 1024], BF16, name="q", tag="q")
                        pt = p5a.tile([128, 1024], BF16, name="pt", tag="pt")
                        w1 = p5a.tile([128, 1024], BF16, name="w1", tag="w1")
                        for b in range(B):
                            sl = slice(b * 512, (b + 1) * 512)
                            col = b * 2 + c
                            nc.vector.tensor_scalar(
                                q[:, sl], t0[:, sl],
                                scalR[:, 24 + xi * 4 + col:25 + xi * 4 + col],
                                scalR[:, 32 + xi * 4 + col:33 + xi * 4 + col],
                                OP.mult, OP.add)
                            nc.vector.tensor_scalar(
                                pt[:, sl], t0[:, sl],
                                scalR[:, 16 + xi * 4 + col:17 + xi * 4 + col],
                                scalR[:, 40 + xi * 4 + col:41 + xi * 4 + col],
                                OP.mult, OP.add)
                            nc.vector.tensor_scalar(
                                w1[:, sl], t1[:, sl],
                                scalR[:, 56 + xi * 4 + col:57 + xi * 4 + col],
                                None, OP.add)
                        if xi == 0:
                            a1 = p5a.tile([128, 1024], BF16, name="a1", tag=f"a1_{c}")
                            nc.vector.tensor_mul(a1[:], q[:], isr[:])
                            a2 = p5a.tile([128, 1024], BF16, name="a2", tag=f"a2_{c}")
                            nc.vector.tensor_mul(a2[:], pt[:], ish[c][:])
                            a3 = p5a.tile([128, 1024], BF16, name="a3", tag=f"a3_{c}")
                            nc.vector.tensor_mul(a3[:], w1[:], pp[:])
                            aj.append((a1, a2, a3))
                        else:
                            # student side: form a_s then diff against teacher
                            srcs = [(q, isr[:]), (pt, ish[c][:]), (w1, pp[:])]
                            for jj in range(3):
                                asrc, mulv = srcs[jj]
                                asb = p5a.tile([128, 1024], BF16, name="as", tag="as")
                                nc.vector.tensor_mul(asb[:], asrc[:], mulv)
                                dj = p5a.tile([128, 1024], BF16, name="dj", tag="dj")
                                nc.vector.tensor_sub(dj[:], asb[:], a_t[c][jj][:])
                                slot = jj * 6 + f * 2 + c
                                if jj == 0:
                                    nc.vector.tensor_reduce(
                                        acc[:, slot:slot + 1], dj[:], AX.X, OP.add,
                                        apply_absolute_value=True)
                                else:
                                    junkB = p5.tile([128, D], BF16, name="junkB",
                                                    tag="junkB", bufs=1)
                                    nc.scalar.activation(junkB[:], dj[:], AF.Abs,
                                                         accum_out=acc[:, slot:slot + 1])
                    if xi == 0:
                        a_t = aj

            # ---------- final ----------
            accr = con.tile([128, 4], F32, name="accr", tag="accr")
            nc.vector.tensor_reduce(accr[:, 0:3],
                                    acc[:, 0:18].rearrange("p (j s) -> p j s", j=3),
                                    AX.X, OP.add)
            nc.vector.memset(accr[:, 3:4], 0.0)
            par = con.tile([128, 4], F32, name="par", tag="par")
            nc.gpsimd.partition_all_reduce(par[:], accr[:], 128,
                                           bass_isa.ReduceOp.add)
            nc.sync.dma_start(out=partials[:], in_=par[0:1, :])


# ---------------- host side ----------------

def bf16(x):
    return np.asarray(x, dtype=ml_dtypes.bfloat16)


def fp8(x):
    return np.asarray(x, dtype=ml_dtypes.float8_e4m3)


def prep_inputs(teacher_feats, student_feats, ref_perm, shared_perm):
    EXTRA_FRAMES = [1, 3, 5, 7]
    tf, sf = np.asarray(teacher_feats), np.asarray(student_feats)
    rp, sp = np.asarray(ref_perm), np.asarray(shared_perm)

    ref = np.stack([tf[:, 0, rp, :], sf[:, 0, rp, :]])          # [2,B,R,D] f32
    ext = np.concatenate([tf[:, f] for f in EXTRA_FRAMES], 1)   # [B,E,D] f32
    sh = np.stack([np.stack([tf[:, t, sp, :], sf[:, s, sp, :]])
                   for s, t in [(1, 2), (2, 4), (3, 6)]])       # [3,2,B,S,D] f32

    extn = ext / np.maximum(np.linalg.norm(ext, axis=-1, keepdims=True), 1e-12)

    def dpack8(x):  # x [..., N, D] f32 -> fp8 [..., 128, DC2*2*N]
        xt = np.swapaxes(x, -1, -2)                             # [..., D, N]
        shp = xt.shape[:-2]
        n = xt.shape[-1]
        xt = xt.reshape(*shp, DC2, 128, 2, n)                   # d = q*256+k*2+i
        xt = np.swapaxes(xt, -4, -3)                            # [..., 128, q, 2, n]
        return np.ascontiguousarray(fp8(xt.reshape(*shp, 128, DC2 * 2 * n)))

    extn_p = dpack8(extn)                                       # [B,128,DC2*2*E]
    refTt_p = dpack8(ref[0])                                    # [B,128,DC2*2*R]
    shT_p = dpack8(sh)                                          # [3,2,B,128,DC2*2*S]

    ss = np.sum(sh.astype(np.float64) * sh, axis=-1)            # [3,2,B,S]
    ss12 = np.zeros((2, 12 * 512), dtype=ml_dtypes.bfloat16)
    ss12[0] = bf16(ss.reshape(-1))
    rrf = np.sum(ref.astype(np.float64) * ref, axis=-1)         # [2,B,R]

    mhalf = np.zeros((2, 128), dtype=ml_dtypes.bfloat16)
    mhalf[0] = -0.5
    offtab = np.broadcast_to((np.arange(32) // 4 * ESH).astype(np.int32),
                             (128, 32)).copy()

    extb = bf16(ext)
    ext8 = fp8(ext)
    in_maps = []
    for c in range(NC_N):
        rs = slice(c * RSH, (c + 1) * RSH)
        esl = slice(c * ESH, (c + 1) * ESH)
        # extTn shard: e-slice within each (q,i) block
        extn_sh = (extn_p.reshape(B, 128, DC2 * 2, E)[:, :, :, esl]
                   .reshape(B, 128, DC2 * 2 * ESH))
        refo = ref[:, :, rs, :]                                  # [2,B,64,D]
        reps = np.concatenate([refo, refo], axis=2)              # [2,B,128,D]
        # reversed-within-64 packing for the sr weights: SwInterleave's
        # column reversal makes u1 psum partition m hold h-row 127-m, whose
        # ref is 63-(m%64)
        refoR = refo[:, :, ::-1, :]
        repsR = np.concatenate([refoR, refoR], axis=2)           # [2,B,128,D]
        refoT = dpack8(repsR)                                    # [2,B,128,DC2*2*128]
        rrep = np.ascontiguousarray(
            np.concatenate([rrf[:, :, rs], rrf[:, :, rs]], axis=2)  # [2,B,128]
            .reshape(4, 128).T.astype(np.float32))               # [128,4] col=xi*2+b
        rrfR = rrf[:, :, rs][:, :, ::-1]
        rrepR = np.ascontiguousarray(
            np.concatenate([rrfR, rrfR], axis=2)
            .reshape(4, 128).T.astype(np.float32))
        jrev = np.fliplr(np.eye(128)).astype(np.float32)
        m = {
            "extTn": np.ascontiguousarray(extn_sh),
            "refTt": refTt_p,
            "refoT": refoT,
            "refnat": bf16(reps),
            "shT": shT_p,
            "extrows": extb,
            "extrows8": ext8,
            "ss12": ss12, "mhalf": mhalf, "offtab": offtab, "rrep": rrep,
            "rrepR": rrepR, "jrev": jrev,
        }
        in_maps.append(m)
    return in_maps


_NC_CACHE = {}


def kernel(teacher_feats, student_feats, ref_perm, shared_perm,
           debug=False, trace=False, use_sim=False):
    key = ("nc", debug)
    if key not in _NC_CACHE:
        _NC_CACHE[key] = build(debug=debug)
    nc = _NC_CACHE[key]
    in_maps = prep_inputs(teacher_feats, student_feats, ref_perm, shared_perm)
    if use_sim:
        from concourse.bass_interp import MultiCoreSim
        nc.insert_bir_kernel_barrier_sem_inc()
        sim = MultiCoreSim(nc, NC_N)
        for t in range(NC_N):
            for name, arr in in_maps[t].items():
                sim.cores[t].tensor(name)[:] = arr
        sim.simulate()
        results = [{"partials": np.array(sim.cores[t].tensor("partials"))}
                   for t in range(NC_N)]

        class _R:
            pass
        res = _R()
        res.results = results
        res.exec_time_ns = None
    else:
        res = run_bass_kernel_spmd(nc, in_maps, list(range(NC_N)), trace=trace)
    parts = np.stack([res.results[c]["partials"][0, :3] for c in range(NC_N)])
    total = B * R * S * K * 3
    loss = np.float32(parts.sum() / total)
    if debug or trace:
        return loss, res
    return loss
